# revision 1
# baseline (speedup 1.0000x reference)
"""Trainium2 Bass kernel for nn_DNNF (segment_reduce DNF network).

Strategy: data-parallel over batch across 8 NeuronCores (1024 rows each).
The literal axis is host-permuted into 12 phase-planes of 896 columns so the
AND segment-sum (depths cycling [2,4,6]) becomes contiguous vector adds, and
the conjunction axis is ordered group/plane-major so the OR segment-sum is
also contiguous adds. GEMM runs in fp16 on the PE (fp32 PSUM accumulate)
with the tanh applied by the Scalar engine during PSUM eviction.
"""
import numpy as np

import concourse.bacc as bacc
import concourse.mybir as mybir
from concourse import bass_utils
from concourse.tile import TileContext

f32 = mybir.dt.float32
fp16 = mybir.dt.float16
AX = mybir.AxisListType
ALU = mybir.AluOpType
ACTF = mybir.ActivationFunctionType

# problem shape (fixed by the harness)
B, D, L, C, F = 8192, 512, 10752, 2688, 256
NCORES = 8
BS = B // NCORES          # rows per core = 1024
NBT = BS // 128           # b-tiles per core = 8
KT = D // 128             # k-tiles = 4
CW = C // 3               # class width = 896 conj per depth-class
PLANES = L // CW          # 12 literal phase-planes
DEPTHS = (2, 4, 6)
PLANE_BASE = {2: 0, 4: 2, 6: 6}
CLS_OFF = {2: 0, 4: 1, 6: 2}
TEMPERATURE = 2.0

_PROGRAM_CACHE = {}


def _derive_structure(lit2conj, conj2form):
    """Validate the expected DNF structure and return group metadata."""
    depths = np.bincount(lit2conj, minlength=C)
    assert np.array_equal(depths, np.tile(np.array(DEPTHS), C // 3)), \
        "unexpected lit2conj structure"
    cpf = np.bincount(conj2form, minlength=F)
    groups = []          # (formula_start, n_formulas, cpf)
    i = 0
    while i < F:
        j = i
        while j < F and cpf[j] == cpf[i]:
            j += 1
        groups.append((i, j - i, int(cpf[i])))
        i = j
    for (_, nf, c_) in groups:
        assert c_ % 3 == 0, "conj-per-formula not divisible by 3"
    cstart = np.concatenate([[0], np.cumsum(cpf)[:-1]])
    assert np.all(cstart % 3 == 0), "formula conj ranges not 3-aligned"
    return groups, cpf, cstart


def _build_permutation(lit2conj, conj2form, groups, cpf, cstart):
    """Map each literal to its (plane, k) column and conj to class/k index.

    k (0..895) within each depth-class is ordered group-major then
    plane-major then formula-major, which makes both the AND adds
    (literal planes) and the OR adds (conj planes) contiguous.
    """
    conj_depth = np.bincount(lit2conj, minlength=C)
    cls = (np.asarray([CLS_OFF[int(d)] for d in conj_depth]))       # [C]
    # group-class offsets in k-space
    gk0 = {}
    acc = 0
    for gi, (f0, nf, c_) in enumerate(groups):
        gk0[gi] = acc
        acc += nf * (c_ // 3)
    assert acc == CW
    group_of_formula = np.zeros(F, np.int64)
    for gi, (f0, nf, c_) in enumerate(groups):
        group_of_formula[f0:f0 + nf] = gi
    # for each conj: its formula, local formula index, plane j within class
    form_of_conj = np.asarray(conj2form, np.int64)
    g_of_conj = group_of_formula[form_of_conj]
    c3 = np.arange(C) // 3
    s3 = (cstart[form_of_conj] // 3).astype(np.int64)
    j_in_form = c3 - s3                                 # plane within class
    f_local = form_of_conj - np.asarray([groups[g][0] for g in g_of_conj])
    k_of_conj = (np.asarray([gk0[g] for g in g_of_conj])
                 + j_in_form * np.asarray([groups[g][1] for g in g_of_conj])
                 + f_local)
    # literal position within its conj
    first_lit = np.concatenate([[0], np.cumsum(conj_depth)[:-1]])
    lpos = np.arange(L) - first_lit[lit2conj]
    plane = np.asarray([PLANE_BASE[int(d)] for d in conj_depth[lit2conj]]) + lpos
    newcol = plane * CW + k_of_conj[lit2conj]
    assert len(np.unique(newcol)) == L
    inv = np.empty(L, np.int64)
    inv[newcol] = np.arange(L)
    return inv, gk0


def _build_program(groups, gk0, bias_zero):
    key = (tuple(groups), tuple(sorted(gk0.items())), bias_zero)
    if key in _PROGRAM_CACHE:
        return _PROGRAM_CACHE[key]
    assert bias_zero, "nonzero literal bias path not implemented"

    nc = bacc.Bacc("TRN2", target_bir_lowering=False, debug=False,
                   num_devices=NCORES)

    xT_d = nc.dram_tensor("xT", [D, BS], f32, kind="ExternalInput").ap()
    wp_d = nc.dram_tensor("wp", [D, L], f32, kind="ExternalInput").ap()
    mp_d = nc.dram_tensor("mp", [D, L], f32, kind="ExternalInput").ap()
    muT_d = nc.dram_tensor("muT", [D, F], f32, kind="ExternalInput").ap()
    mun_d = nc.dram_tensor("mun", [F, D], f32, kind="ExternalInput").ap()
    sig_d = nc.dram_tensor("sig", [F], f32, kind="ExternalInput").ap()
    eye_d = nc.dram_tensor("eye", [128, 128], fp16, kind="ExternalInput").ap()
    out_d = nc.dram_tensor("out", [BS, F], f32, kind="ExternalOutput").ap()
    scr_d = nc.dram_tensor("m2scr", [F], f32, kind="Internal").ap()

    LN_T = float(np.log(TEMPERATURE))

    with TileContext(nc) as tc:
        with tc.tile_pool(name="cst", bufs=1) as cst, \
             tc.tile_pool(name="stg", bufs=4) as stg, \
             tc.tile_pool(name="stgw", bufs=6) as stgw, \
             tc.tile_pool(name="wrk", bufs=2) as wrk, \
             tc.tile_pool(name="tail", bufs=1) as tail, \
             tc.tile_pool(name="pp", bufs=3) as ppool, \
             tc.tile_pool(name="ps", bufs=2, space="PSUM") as psp:

            # ---------- constants / prep ----------
            bias_cols = {}

            def bias_col(val):
                v = float(val)
                if v not in bias_cols:
                    t = cst.tile([128, 1], f32, tag=f"bc{len(bias_cols)}")
                    nc.vector.memset(t[:], v)
                    bias_cols[v] = t
                return bias_cols[v][:]

            xT_h = cst.tile([128, KT, BS], fp16, tag="xTh")
            muT_h = cst.tile([128, KT, F], fp16, tag="muTh")
            # ---------- pipelined Wm-chunk build + literals ----------
            # chunk-pair outer, b-tile inner: W/M DMA streams under the
            # whole GEMM phase; Wm chunks are transient (never resident).
            conj_all = tail.tile([128, NBT, C], fp16, tag="conj_all")
            form_all = tail.tile([128, NBT, F], f32, tag="form_all")
            acc = {b: tail.tile([128, CW], fp16, tag=f"acc{b}",
                                 name=f"acc{b}")
                   for b in range(NBT)}
            partials = {}
            NC2 = PLANES // 2
            wm_tiles = {}

            def build_chunk(c2):
                t = wrk.tile([128, KT, 2, CW], fp16, tag="wmch")
                for half in range(2):
                    pl = 2 * c2 + half
                    cs = slice(pl * CW, (pl + 1) * CW)
                    for k in range(KT):
                        wst = stgw.tile([128, 1024], f32, tag="stgw")
                        nc.sync.dma_start(wst[:, 0:CW],
                                          wp_d[k * 128:(k + 1) * 128, cs])
                        mst = stgw.tile([128, 1024], f32, tag="stgw")
                        nc.sync.dma_start(mst[:, 0:CW],
                                          mp_d[k * 128:(k + 1) * 128, cs])
                        nc.vector.tensor_mul(t[:, k, half, :], wst[:, 0:CW],
                                             mst[:, 0:CW])
                wm_tiles[c2] = t

            build_chunk(0)
            eye_t = cst.tile([128, 128], fp16, tag="eye")
            nc.gpsimd.dma_start(eye_t[:], eye_d[:])

            # xT: load f32 staging per k-tile, cast to fp16
            for k in range(KT):
                st = stg.tile([128, 1024], f32, tag="stg")
                nc.sync.dma_start(st[:, 0:BS], xT_d[k * 128:(k + 1) * 128, :])
                nc.vector.tensor_copy(xT_h[:, k, :], st[:, 0:BS])
            # muT
            for k in range(KT):
                st = stg.tile([128, 1024], f32, tag="stg")
                nc.gpsimd.dma_start(st[:, 0:F], muT_d[k * 128:(k + 1) * 128, :])
                nc.vector.tensor_copy(muT_h[:, k, :], st[:, 0:F])

            # m2 = sum(mu^2) per formula  ->  DRAM scratch -> row [1, F]
            m2col = cst.tile([128, F // 128], f32, tag="m2col")
            for t in range(F // 128):
                st = stg.tile([128, 1024], f32, tag="stg")
                nc.gpsimd.dma_start(st[:, 0:D], mun_d[t * 128:(t + 1) * 128, :])
                sq = stg.tile([128, 1024], f32, tag="stg")
                nc.vector.tensor_mul(sq[:, 0:D], st[:, 0:D], st[:, 0:D])
                nc.vector.reduce_sum(m2col[:, t:t + 1], sq[:, 0:D],
                                     axis=AX.XYZW)
                nc.gpsimd.dma_start(scr_d[t * 128:(t + 1) * 128],
                                  m2col[:, t:t + 1])
            m2row = cst.tile([1, F], f32, tag="m2row")
            nc.gpsimd.dma_start(m2row[:], scr_d[None, :])

            sigrow = cst.tile([1, F], f32, tag="sigrow")
            nc.gpsimd.dma_start(sigrow[:], sig_d[None, :])
            s2row = cst.tile([1, F], f32, tag="s2row")
            nc.vector.tensor_mul(s2row[:], sigrow[:], sigrow[:])
            arow = cst.tile([1, F], f32, tag="arow")
            nc.vector.reciprocal(arow[:], s2row[:])
            # beta = -0.5*m2 + ln(T)*sigma^2   (so that a*(G+beta-0.5sq)
            #   = a*(G-0.5sq-0.5m2) + ln T)
            t1 = cst.tile([1, F], f32, tag="t1row")
            nc.vector.tensor_scalar_mul(t1[:], m2row[:], -0.5)
            t2 = cst.tile([1, F], f32, tag="t2row")
            nc.vector.tensor_scalar_mul(t2[:], s2row[:], LN_T)
            brow = cst.tile([1, F], f32, tag="brow")
            nc.vector.tensor_add(brow[:], t1[:], t2[:])
            # hi/lo fp16 splits of beta and a
            bhi = cst.tile([1, F], fp16, tag="bhi")
            nc.vector.tensor_copy(bhi[:], brow[:])
            blo32 = cst.tile([1, F], f32, tag="blo32")
            nc.vector.tensor_sub(blo32[:], brow[:], bhi[:])
            blo = cst.tile([1, F], fp16, tag="blo")
            nc.vector.tensor_copy(blo[:], blo32[:])
            ahi = cst.tile([1, F], fp16, tag="ahi")
            nc.vector.tensor_copy(ahi[:], arow[:])
            alo32 = cst.tile([1, F], f32, tag="alo32")
            nc.vector.tensor_sub(alo32[:], arow[:], ahi[:])
            alo = cst.tile([1, F], fp16, tag="alo")
            nc.vector.tensor_copy(alo[:], alo32[:])
            # beta2 / a2: [2, F] fp16 via tiny DMA through scratch is
            # avoided — use two K=1 matmuls instead (ones row x row).
            ones1 = cst.tile([1, 128], fp16, tag="ones1")
            nc.vector.memset(ones1[:], 1.0)


            for c2 in range(NC2):
                if c2 + 1 < NC2:
                    build_chunk(c2 + 1)
                wmc = wm_tiles.pop(c2)
                for b in range(NBT):
                    bs = slice(b * 128, (b + 1) * 128)
                    ps_l = psp.tile([128, 2048], f32, tag="ps")
                    for half in range(2):
                        for (o0, w_) in ((0, 512), (512, 384)):
                            po = half * 1024 + o0
                            for k in range(KT):
                                nc.tensor.matmul(
                                    ps_l[:, po:po + w_], xT_h[:, k, bs],
                                    wmc[:, k, half, o0:o0 + w_],
                                    start=(k == 0), stop=(k == KT - 1))
                    ev = ppool.tile([128, 2, CW], fp16, tag="ev")
                    pv = ps_l[:].rearrange("p (h w) -> p h w", h=2)
                    nc.scalar.activation(ev[:], pv[:, :, 0:CW], ACTF.Tanh)
                    # incremental AND-stage adds per depth-class
                    if c2 == 0:
                        nc.vector.tensor_add(conj_all[:, b, 0:CW],
                                             ev[:, 0, :], ev[:, 1, :])
                    elif c2 == 1:
                        nc.vector.tensor_add(acc[b][:], ev[:, 0, :],
                                             ev[:, 1, :])
                    elif c2 == 2:
                        t2 = ppool.tile([128, CW], fp16, tag="evs")
                        nc.vector.tensor_add(t2[:], ev[:, 0, :], ev[:, 1, :])
                        nc.vector.tensor_add(conj_all[:, b, CW:2 * CW],
                                             acc[b][:], t2[:])
                    elif c2 == 3:
                        nc.vector.tensor_add(acc[b][:], ev[:, 0, :],
                                             ev[:, 1, :])
                    elif c2 == 4:
                        t2 = ppool.tile([128, CW], fp16, tag="evs")
                        nc.vector.tensor_add(t2[:], ev[:, 0, :], ev[:, 1, :])
                        nc.vector.tensor_add(acc[b][:], acc[b][:], t2[:])
                    else:
                        t2 = ppool.tile([128, CW], fp16, tag="evs")
                        nc.vector.tensor_add(t2[:], ev[:, 0, :], ev[:, 1, :])
                        nc.vector.tensor_add(conj_all[:, b, 2 * CW:3 * CW],
                                             acc[b][:], t2[:])
                    # conj tanh once a class is complete for this b
                    for ci, (c2done, d) in enumerate(zip((0, 2, 5), DEPTHS)):
                        if c2 == c2done:
                            sl = conj_all[:, b, ci * CW:(ci + 1) * CW]
                            nc.scalar.activation(sl, sl, ACTF.Tanh,
                                                 bias=bias_col(1.5 - d))
                # form partials for the class that just completed (all b)
                ci = {0: 0, 2: 1, 5: 2}.get(c2)
                if ci is not None:
                    for gi, (f0, nf, cpf_g) in enumerate(groups):
                        m = cpf_g // 3
                        k0 = ci * CW + gk0[gi]
                        sl = [conj_all[:, :, k0 + j * nf:k0 + (j + 1) * nf]
                              for j in range(m)]
                        p = tail.tile([128, NBT, 64], fp16,
                                      tag=f"pt{gi}_{ci}")
                        pv_ = p[:, :, 0:nf]
                        if m == 2:
                            nc.vector.tensor_add(pv_, sl[0], sl[1])
                        elif m == 3:
                            nc.vector.tensor_add(pv_, sl[0], sl[1])
                            nc.vector.tensor_add(pv_, pv_, sl[2])
                        elif m == 4:
                            tmp = ppool.tile([128, NBT, 64], fp16, tag="fpt")
                            tmpv = tmp[:, :, 0:nf]
                            nc.vector.tensor_add(pv_, sl[0], sl[1])
                            nc.vector.tensor_add(tmpv, sl[2], sl[3])
                            nc.vector.tensor_add(pv_, pv_, tmpv)
                        elif m == 5:
                            tmp = ppool.tile([128, NBT, 64], fp16, tag="fpt")
                            tmpv = tmp[:, :, 0:nf]
                            nc.vector.tensor_add(pv_, sl[0], sl[1])
                            nc.vector.tensor_add(tmpv, sl[2], sl[3])
                            nc.vector.tensor_add(pv_, pv_, tmpv)
                            nc.vector.tensor_add(pv_, pv_, sl[4])
                        else:
                            raise AssertionError(f"unsupported cpf {cpf_g}")
                        partials[(gi, ci)] = pv_

                if c2 == 2:
                    # A_bc = broadcast of a (1/sigma^2) to [128, F] via 2 rank-1 mms
                    ps_bc = psp.tile([128, 2048], f32, tag="ps")
                    nc.tensor.matmul(ps_bc[:, 0:F], ones1[:], ahi[:],
                                     start=True, stop=False)
                    nc.tensor.matmul(ps_bc[:, 0:F], ones1[:], alo[:],
                                     start=False, stop=True)
                    a_bc = cst.tile([128, F], f32, tag="a_bc")
                    nc.vector.tensor_copy(a_bc[:], ps_bc[:, 0:F])

                    # ---------- loc branch: G2 psums + Gram diag ----------
                    sq_all = cst.tile([128, NBT], f32, tag="sq_all")
                    sqh_all = cst.tile([128, NBT], f32, tag="sqh_all")
                    z_all = tail.tile([128, NBT, F], fp16, tag="z_all")
                    for b in range(NBT):
                        bs = slice(b * 128, (b + 1) * 128)
                        # Gram diag for ||x||^2
                        ps_g = psp.tile([128, 2048], f32, tag="ps")
                        for k in range(KT):
                            nc.tensor.matmul(ps_g[:, 0:128], xT_h[:, k, bs],
                                             xT_h[:, k, bs],
                                             start=(k == 0), stop=(k == KT - 1))
                        gd = stg.tile([128, 1024], f32, tag="stg")
                        nc.vector.tensor_mul(gd[:, 0:128], ps_g[:, 0:128], eye_t[:])
                        nc.vector.reduce_sum(sq_all[:, b:b + 1], gd[:, 0:128],
                                             axis=AX.XYZW)
                        nc.vector.tensor_scalar_mul(sqh_all[:, b:b + 1],
                                                    sq_all[:, b:b + 1], 0.5)
                        # G2 + rank-1 beta
                        ps_G = psp.tile([128, 2048], f32, tag="ps")
                        for k in range(KT):
                            nc.tensor.matmul(ps_G[:, 0:F], xT_h[:, k, bs],
                                             muT_h[:, k, :],
                                             start=(k == 0), stop=False)
                        nc.tensor.matmul(ps_G[:, 0:F], ones1[:], bhi[:],
                                         start=False, stop=False)
                        nc.tensor.matmul(ps_G[:, 0:F], ones1[:], blo[:],
                                         start=False, stop=True)
                        # z = a * (G + beta - 0.5*||x||^2)   [-> T*exp(w) after Exp]
                        nc.vector.scalar_tensor_tensor(
                            z_all[:, b, :], ps_G[:, 0:F], sqh_all[:, b:b + 1],
                            a_bc[:], op0=ALU.subtract, op1=ALU.mult)

                    e_t = tail.tile([128, NBT, F], fp16, tag="e_t")
                    nc.scalar.activation(z_all[:], z_all[:], ACTF.Exp)
                    nc.scalar.activation(e_t[:], z_all[:], ACTF.Exp)
                    s_t = tail.tile([128, NBT], f32, tag="s_t")
                    nc.vector.reduce_sum(s_t[:], e_t[:], axis=AX.X)
                    r_t = tail.tile([128, NBT], f32, tag="r_t")
                    nc.vector.reciprocal(r_t[:], s_t[:])


            # ---------- combine partials + dnnf ----------
            for gi, (f0, nf, cpf_g) in enumerate(groups):
                fv = form_all[:, :, f0:f0 + nf]
                tmpf = ppool.tile([128, NBT, 64], f32, tag="fpf32")
                tmpfv = tmpf[:, :, 0:nf]
                nc.vector.tensor_add(tmpfv, partials[(gi, 0)],
                                     partials[(gi, 1)])
                nc.vector.tensor_add(fv, tmpfv, partials[(gi, 2)])
                nc.scalar.activation(fv, fv, ACTF.Tanh,
                                     bias=bias_col(cpf_g - 1.5))

            # ---------- softmax tail (exp/sum/recip ran mid-loop) ----------
            nc.vector.tensor_mul(form_all[:], form_all[:], e_t[:])
            for b in range(NBT):
                nc.vector.tensor_scalar_mul(form_all[:, b, :],
                                            form_all[:, b, :],
                                            r_t[:, b:b + 1])
            nc.sync.dma_start(out_d.rearrange("(b p) f -> p b f", p=128),
                              form_all[:])

    nc.compile()
    _PROGRAM_CACHE[key] = nc
    return nc


def kernel(x, weight, learnable_binary_mask, bias, mu, sigma,
           lit2conj, conj2form):
    x = np.asarray(x, np.float32)
    weight = np.asarray(weight, np.float32)
    mask = np.asarray(learnable_binary_mask, np.float32)
    bias = np.asarray(bias, np.float32)
    mu = np.asarray(mu, np.float32)
    sigma = np.asarray(sigma, np.float32)
    lit2conj = np.asarray(lit2conj, np.int64)
    conj2form = np.asarray(conj2form, np.int64)

    groups, cpf, cstart = _derive_structure(lit2conj, conj2form)
    inv, gk0 = _build_permutation(lit2conj, conj2form, groups, cpf, cstart)
    bias_zero = bool(np.all(bias == 0))

    nc = _build_program(groups, gk0, bias_zero)

    wp = np.ascontiguousarray(weight[:, inv])
    mp = np.ascontiguousarray(mask[:, inv])
    muT = np.ascontiguousarray(mu.T)
    eye = np.eye(128, dtype=np.float16)

    in_maps = []
    for i in range(NCORES):
        xs = x[i * BS:(i + 1) * BS]
        in_maps.append({
            "xT": np.ascontiguousarray(xs.T),
            "wp": wp, "mp": mp, "muT": muT, "mun": mu,
            "sig": sigma, "eye": eye,
        })

    res = bass_utils.run_bass_kernel_spmd(nc, in_maps,
                                          core_ids=list(range(NCORES)))
    out = np.concatenate([res.results[i]["out"] for i in range(NCORES)],
                         axis=0)
    return out.astype(np.float32)



# revision 7
# speedup vs baseline: 1.0192x; 1.0192x over previous
"""Trainium2 Bass kernel for nn_DNNF (segment_reduce DNF network).

Strategy: data-parallel over batch across 8 NeuronCores (1024 rows each).
Literal columns are host-permuted into 12 phase-planes of 896 so the AND
segment-sum becomes contiguous slab adds and the OR segment-sum contiguous
strided adds.  The literal GEMM runs in fp8 (e4m3) DoubleRow mode: plain
fp8 for the tanh-saturated depth-4/6 planes, hi+lo compensated fp8 for the
sensitive depth-2 planes (processed last so the PE-heavy burst overlaps the
scalar engine's conj/exp tail).  W and M stream in as fp16 and are fused
(multiply+fp8-cast) on Vector/GpSimd.  Scalar engine applies every tanh on
PSUM eviction.
"""
import numpy as np
import ml_dtypes

import concourse.bacc as bacc
import concourse.mybir as mybir
from concourse import bass_utils
from concourse.tile import TileContext

f32 = mybir.dt.float32
fp16 = mybir.dt.float16
fp8 = mybir.dt.float8e4
AX = mybir.AxisListType
ALU = mybir.AluOpType
ACTF = mybir.ActivationFunctionType
DR = mybir.MatmulPerfMode.DoubleRow

# problem shape (fixed by the harness)
B, D, L, C, F = 8192, 512, 10752, 2688, 256
NCORES = 8
BS = B // NCORES          # rows per core = 1024
NBT = BS // 128           # b-tiles per core = 8
KT = D // 128             # k-tiles = 4
CW = C // 3               # class width = 896 conj per depth-class
PLANES = L // CW          # 12 literal phase-planes
DEPTHS = (2, 4, 6)
PLANE_BASE = {2: 0, 4: 2, 6: 6}
CLS_OFF = {2: 0, 4: 1, 6: 2}
TEMPERATURE = 2.0
XS = 32.0                 # x fp8 scale
WS = 2048.0               # w fp16/fp8 scale
INV_SCALE = 1.0 / (XS * WS)

# plane processing order: depth-4, depth-6, then the PE-heavy compensated
# depth-2 planes last (overlaps scalar-engine conj/exp tail work)
ORDER = [2, 3, 4, 5, 6, 7, 8, 9, 10, 11, 0, 1]
CLASS_OF_PLANE = {0: 0, 1: 0, 2: 1, 3: 1, 4: 1, 5: 1,
                  6: 2, 7: 2, 8: 2, 9: 2, 10: 2, 11: 2}
FIRST_OF_CLASS = {0: 0, 1: 2, 2: 6}
LAST_OF_CLASS = {0: 1, 1: 5, 2: 11}

_PROGRAM_CACHE = {}


def _derive_structure(lit2conj, conj2form):
    """Validate the expected DNF structure and return group metadata."""
    depths = np.bincount(lit2conj, minlength=C)
    assert np.array_equal(depths, np.tile(np.array(DEPTHS), C // 3)), \
        "unexpected lit2conj structure"
    cpf = np.bincount(conj2form, minlength=F)
    groups = []          # (formula_start, n_formulas, cpf)
    i = 0
    while i < F:
        j = i
        while j < F and cpf[j] == cpf[i]:
            j += 1
        groups.append((i, j - i, int(cpf[i])))
        i = j
    for (_, nf, c_) in groups:
        assert c_ % 3 == 0, "conj-per-formula not divisible by 3"
    cstart = np.concatenate([[0], np.cumsum(cpf)[:-1]])
    assert np.all(cstart % 3 == 0), "formula conj ranges not 3-aligned"
    return groups, cpf, cstart


def _build_permutation(lit2conj, conj2form, groups, cpf, cstart):
    """Map each literal to its (plane, k) column; see baseline docstring."""
    conj_depth = np.bincount(lit2conj, minlength=C)
    gk0 = {}
    acc = 0
    for gi, (f0, nf, c_) in enumerate(groups):
        gk0[gi] = acc
        acc += nf * (c_ // 3)
    assert acc == CW
    group_of_formula = np.zeros(F, np.int64)
    for gi, (f0, nf, c_) in enumerate(groups):
        group_of_formula[f0:f0 + nf] = gi
    form_of_conj = np.asarray(conj2form, np.int64)
    g_of_conj = group_of_formula[form_of_conj]
    c3 = np.arange(C) // 3
    s3 = (cstart[form_of_conj] // 3).astype(np.int64)
    j_in_form = c3 - s3
    f_local = form_of_conj - np.asarray([groups[g][0] for g in g_of_conj])
    k_of_conj = (np.asarray([gk0[g] for g in g_of_conj])
                 + j_in_form * np.asarray([groups[g][1] for g in g_of_conj])
                 + f_local)
    first_lit = np.concatenate([[0], np.cumsum(conj_depth)[:-1]])
    lpos = np.arange(L) - first_lit[lit2conj]
    plane = np.asarray([PLANE_BASE[int(d)] for d in conj_depth[lit2conj]]) + lpos
    newcol = plane * CW + k_of_conj[lit2conj]
    assert len(np.unique(newcol)) == L
    inv = np.empty(L, np.int64)
    inv[newcol] = np.arange(L)
    return inv, gk0


def _build_program(groups, gk0, bias_zero):
    key = (tuple(groups), tuple(sorted(gk0.items())), bias_zero)
    if key in _PROGRAM_CACHE:
        return _PROGRAM_CACHE[key]
    assert bias_zero, "nonzero literal bias path not implemented"

    LN_T = float(np.log(TEMPERATURE))

    nc = bacc.Bacc("TRN2", target_bir_lowering=False, debug=False,
                   num_devices=NCORES)

    # dram inputs (host pre-arranged to sbuf layouts; see prepare())
    xhl_d = nc.dram_tensor("xhl", [128, 3 * KT * BS], fp8,
                           kind="ExternalInput").ap()
    w16_d = nc.dram_tensor("w16", [128, PLANES, KT, CW], fp16,
                           kind="ExternalInput").ap()
    m16_d = nc.dram_tensor("m16", [128, PLANES, KT, CW], fp16,
                           kind="ExternalInput").ap()
    mu8_d = nc.dram_tensor("mu8", [128, 2 * 2 * F], fp8,
                           kind="ExternalInput").ap()
    mun_d = nc.dram_tensor("mun", [128, 2 * D], f32,
                           kind="ExternalInput").ap()
    sig_d = nc.dram_tensor("sig", [F], f32, kind="ExternalInput").ap()
    eye_d = nc.dram_tensor("eye", [128, 128], fp16, kind="ExternalInput").ap()
    out_d = nc.dram_tensor("out", [128, NBT * F], f32,
                           kind="ExternalOutput").ap()
    scr_d = nc.dram_tensor("bscr", [F], f32, kind="Internal").ap()
    scr2_d = nc.dram_tensor("ascr", [F], f32, kind="Internal").ap()

    with TileContext(nc) as tc:
        with tc.tile_pool(name="cst", bufs=1) as cst, \
             tc.tile_pool(name="wst", bufs=2) as wpool, \
             tc.tile_pool(name="mst", bufs=2) as mpool, \
             tc.tile_pool(name="cur", bufs=2) as curp, \
             tc.tile_pool(name="wm16p", bufs=1) as wm16p, \
             tc.tile_pool(name="ps", bufs=3, space="PSUM") as psp, \
             tc.tile_pool(name="psz", bufs=2, space="PSUM") as psz:

            bias_cols = {}

            def bias_col(val):
                v = float(val)
                if v not in bias_cols:
                    t = cst.tile([128, 1], f32, tag=f"bc{len(bias_cols)}")
                    nc.vector.memset(t[:], v)
                    bias_cols[v] = t
                return bias_cols[v][:]

            # ---------- resident tiles ----------
            xhl = cst.tile([128, 3, KT, BS], fp8, tag="xhl")
            nc.sync.dma_start(
                xhl[:].rearrange("p a b c -> p (a b c)"), xhl_d[:, :])
            wm8 = cst.tile([128, KT, PLANES - 2, CW], fp8, tag="wm8")
            wmd2 = cst.tile([128, KT, 2, 2 * CW], fp8, tag="wmd2")
            conj0 = cst.tile([128, NBT, CW], fp16, tag="conj0")
            conj1 = cst.tile([128, NBT, CW], fp16, tag="conj1")
            conj2 = cst.tile([128, NBT, CW], fp16, tag="conj2")
            conj_of = {0: conj0, 1: conj1, 2: conj2}
            por = cst.tile([128, 3, NBT, F], fp16, tag="por")
            form = cst.tile([128, NBT, F], f32, tag="form")
            z_all = cst.tile([128, NBT, F], fp16, tag="z_all")
            e_t = z_all  # Exp applied twice in place; z dead after
            s_t = cst.tile([128, NBT], f32, tag="s_t")
            r_t = cst.tile([128, NBT], f32, tag="r_t")
            mu8_t = cst.tile([128, 2, 2, F], fp8, tag="mu8")
            nc.sync.dma_start(
                mu8_t[:].rearrange("p a b c -> p (a b c)"), mu8_d[:, :])
            eye_t = cst.tile([128, 128], fp16, tag="eye")
            nc.sync.dma_start(eye_t[:], eye_d[:])

            # ---------- loc constants: m2, beta row, a' row ----------
            mun_t = cst.tile([128, 2, D], f32, tag="mun")
            nc.sync.dma_start(
                mun_t[:].rearrange("p a b -> p (a b)"), mun_d[:, :])
            nc.vector.tensor_mul(mun_t[:], mun_t[:], mun_t[:])
            m2c = cst.tile([128, 2], f32, tag="m2c")
            nc.vector.reduce_sum(m2c[:], mun_t[:], axis=AX.X)
            sigc = cst.tile([128, 2], f32, tag="sigc")
            nc.sync.dma_start(sigc[:], sig_d.rearrange("(a p) -> p a", p=128))
            s2c = cst.tile([128, 2], f32, tag="s2c")
            nc.vector.tensor_mul(s2c[:], sigc[:], sigc[:])
            # beta = -0.5*m2 + ln(T)*sigma^2 ; store 32*beta as fp16 row
            bc1 = cst.tile([128, 2], f32, tag="bc1")
            nc.vector.tensor_scalar_mul(bc1[:], m2c[:], -0.5 * XS)
            bc2 = cst.tile([128, 2], f32, tag="bc2")
            nc.vector.tensor_scalar_mul(bc2[:], s2c[:], LN_T * XS)
            bcol = cst.tile([128, 2], f32, tag="bcol")
            nc.vector.tensor_add(bcol[:], bc1[:], bc2[:])
            nc.sync.dma_start(scr_d.rearrange("(a p) -> p a", p=128), bcol[:])
            brow = cst.tile([1, F], f32, tag="brow")
            nc.sync.dma_start(brow[:], scr_d[None, :])
            brow16 = cst.tile([1, F], fp16, tag="brow16")
            nc.vector.tensor_copy(brow16[:], brow[:])
            # a' = 1/(sigma^2 * XS^2) as fp16 row (z scale absorbed)
            acol = cst.tile([128, 2], f32, tag="acol")
            nc.vector.reciprocal(acol[:], s2c[:])
            acol2 = cst.tile([128, 2], f32, tag="acol2")
            nc.vector.tensor_scalar_mul(acol2[:], acol[:], 1.0 / (XS * XS))
            nc.sync.dma_start(scr2_d.rearrange("(a p) -> p a", p=128),
                              acol2[:])
            arow = cst.tile([1, F], f32, tag="arow")
            nc.sync.dma_start(arow[:], scr2_d[None, :])
            arow16 = cst.tile([1, F], fp16, tag="arow16")
            nc.vector.tensor_copy(arow16[:], arow[:])
            ones1 = cst.tile([1, 128], fp16, tag="ones1")
            nc.vector.memset(ones1[:], 1.0)
            o32 = cst.tile([1, 128], fp16, tag="o32")
            nc.vector.memset(o32[:], XS)

            # a' broadcast to [128, F] via rank-1 matmul
            ps_a = psz.tile([128, 512], f32, tag="psz")
            nc.tensor.matmul(ps_a[:, 0:F], ones1[:], arow16[:],
                             start=True, stop=True)
            a_bc = cst.tile([128, F], f32, tag="a_bc")
            nc.vector.tensor_copy(a_bc[:], ps_a[:, 0:F])

            # ---------- helpers ----------
            def naive_lhsT(q, b):
                # slots = k-tiles (2q, 2q+1) of x-hi
                return xhl[:, 0, 2 * q:2 * q + 2, b * 128:(b + 1) * 128]

            def comp_lhsT(k, b, rev):
                # slots = (hi, lo) or (lo, hi) at k-tile k
                return xhl[:, rev:rev + 2, k, b * 128:(b + 1) * 128]

            def do_plane(p):
                ci = CLASS_OF_PLANE[p]
                d2 = (ci == 0)
                wt = wpool.tile([128, KT, CW], fp16, tag="w")
                nc.sync.dma_start(
                    wt[:].rearrange("p a b -> p (a b)"),
                    w16_d[:, p].rearrange("p a b -> p (a b)"))
                mt = mpool.tile([128, KT, CW], fp16, tag="m")
                nc.sync.dma_start(
                    mt[:].rearrange("p a b -> p (a b)"),
                    m16_d[:, p].rearrange("p a b -> p (a b)"))
                if d2:
                    # two-step: fp16 product (for the residual), then casts
                    pid = p  # 0 or 1
                    wm16 = wm16p.tile([128, KT, CW], fp16, tag="wm16")
                    nc.vector.tensor_mul(wm16[:, 0:2], wt[:, 0:2], mt[:, 0:2])
                    nc.gpsimd.tensor_mul(wm16[:, 2:4], wt[:, 2:4], mt[:, 2:4])
                    hi = wmd2[:, :, 0, pid * CW:(pid + 1) * CW]
                    lo = wmd2[:, :, 1, pid * CW:(pid + 1) * CW]
                    nc.vector.tensor_copy(hi, wm16[:])
                    nc.vector.tensor_sub(lo, wm16[:], hi)
                else:
                    dst = wm8[:, :, p - 2, :]
                    nc.vector.tensor_mul(dst[:, 0:1], wt[:, 0:1], mt[:, 0:1])
                    nc.gpsimd.tensor_mul(dst[:, 1:4], wt[:, 1:4], mt[:, 1:4])

                first = (p == FIRST_OF_CLASS[ci])
                target = conj_of[ci] if first else curp.tile(
                    [128, NBT, CW], fp16, tag="cur")
                for b in range(NBT):
                    ps = psp.tile([128, CW], f32, tag="ps")
                    if d2:
                        for co in (0, 448):
                            for k in range(KT):
                                rhs = wmd2[:, k, :,
                                           p * CW + co:p * CW + co + 448]
                                nc.tensor.matmul(
                                    ps[:, co:co + 448], comp_lhsT(k, b, 0),
                                    rhs, start=(k == 0), stop=False,
                                    perf_mode=DR)
                                nc.tensor.matmul(
                                    ps[:, co:co + 448], comp_lhsT(k, b, 1),
                                    rhs, start=False, stop=(k == KT - 1),
                                    perf_mode=DR)
                    else:
                        for co in (0, 448):
                            for q in range(2):
                                nc.tensor.matmul(
                                    ps[:, co:co + 448], naive_lhsT(q, b),
                                    wm8[:, 2 * q:2 * q + 2, p - 2,
                                        co:co + 448],
                                    start=(q == 0), stop=(q == 1),
                                    perf_mode=DR)
                    nc.scalar.activation(target[:, b, :], ps[:],
                                         ACTF.Tanh, scale=INV_SCALE)
                if not first:
                    nc.vector.tensor_add(conj_of[ci][:], conj_of[ci][:],
                                         target[:])
                if p == LAST_OF_CLASS[ci]:
                    close_class(ci)

            def close_class(ci):
                d = DEPTHS[ci]
                cj = conj_of[ci]
                nc.scalar.activation(cj[:], cj[:], ACTF.Tanh,
                                     bias=bias_col(1.5 - d))
                # OR partial trees into por[ci]
                for gi, (f0, nf, cpf_g) in enumerate(groups):
                    mg = cpf_g // 3
                    k0 = gk0[gi]
                    dst = por[:, ci, :, f0:f0 + nf]
                    nc.vector.tensor_add(dst, cj[:, :, k0:k0 + nf],
                                         cj[:, :, k0 + nf:k0 + 2 * nf])
                    for j in range(2, mg):
                        nc.vector.tensor_add(
                            dst, dst, cj[:, :, k0 + j * nf:k0 + (j + 1) * nf])

            def do_loc():
                # z = a' * (G_raw + beta_raw - 0.5*||xs||^2) ; raw = XS^2-scaled
                for b in range(NBT):
                    bsl = slice(b * 128, (b + 1) * 128)
                    ps_g = psz.tile([128, 512], f32, tag="psz")
                    for q in range(2):
                        nc.tensor.matmul(ps_g[:, 0:128],
                                         naive_lhsT(q, b), xhl[:, 0, 2 * q:2 * q + 2, bsl],
                                         start=(q == 0), stop=(q == 1),
                                         perf_mode=DR)
                    gd = cst.tile([128, 128], f32, tag=f"gd{b % 2}")
                    gdv = gd[:]
                    nc.vector.tensor_mul(gdv, ps_g[:, 0:128], eye_t[:])
                    sqc = cst.tile([128, 1], f32, tag=f"sq{b}")
                    nc.vector.reduce_sum(sqc[:], gdv, axis=AX.XYZW)
                    sqh = cst.tile([128, 1], f32, tag=f"sqh{b}")
                    nc.vector.tensor_scalar_mul(sqh[:], sqc[:], 0.5)

                    ps_z = psz.tile([128, 512], f32, tag="psz")
                    for q in range(2):
                        nc.tensor.matmul(ps_z[:, 0:F], naive_lhsT(q, b),
                                         mu8_t[:, q], start=(q == 0),
                                         stop=False, perf_mode=DR)
                    nc.tensor.matmul(ps_z[:, 0:F], o32[:], brow16[:],
                                     start=False, stop=True,
                                     skip_group_check=True)
                    nc.vector.scalar_tensor_tensor(
                        z_all[:, b, :], ps_z[:, 0:F], sqh[:], a_bc[:],
                        op0=ALU.subtract, op1=ALU.mult)

            def finish():
                # z -> T*e^w -> softmax pieces
                nc.scalar.activation(z_all[:], z_all[:], ACTF.Exp, bias=0.0)
                nc.scalar.activation(e_t[:], z_all[:], ACTF.Exp, bias=0.0)
                nc.vector.reduce_sum(s_t[:], e_t[:], axis=AX.X)
                nc.vector.reciprocal(r_t[:], s_t[:])
                # form = sum of class partials (f32), tanh with per-group bias
                nc.vector.tensor_add(form[:], por[:, 0], por[:, 1])
                nc.vector.tensor_add(form[:], form[:], por[:, 2])
                for gi, (f0, nf, cpf_g) in enumerate(groups):
                    fv = form[:, :, f0:f0 + nf]
                    nc.scalar.activation(fv, fv, ACTF.Tanh,
                                         bias=bias_col(cpf_g - 1.5))
                nc.vector.tensor_mul(form[:], form[:], e_t[:])
                for b in range(NBT):
                    nc.gpsimd.tensor_scalar_mul(form[:, b, :], form[:, b, :],
                                                r_t[:, b:b + 1])
                nc.sync.dma_start(
                    out_d[:, :], form[:].rearrange("p a b -> p (a b)"))

            # ---------- schedule ----------
            for i, p in enumerate(ORDER):
                do_plane(p)
                if i == 1:
                    do_loc()
            finish()

    nc.compile()
    _PROGRAM_CACHE[key] = nc
    return nc


def _q8(a):
    return np.clip(a, -240.0, 240.0).astype(ml_dtypes.float8_e4m3fn)


def prepare(inputs):
    """Host-side shard/layout prep. Returns (nc, in_maps)."""
    x = np.asarray(inputs["x"], np.float32)
    weight = np.asarray(inputs["weight"], np.float32)
    mask = np.asarray(inputs["learnable_binary_mask"], np.float32)
    bias = np.asarray(inputs["bias"], np.float32)
    mu = np.asarray(inputs["mu"], np.float32)
    sigma = np.asarray(inputs["sigma"], np.float32)
    lit2conj = np.asarray(inputs["lit2conj"], np.int64)
    conj2form = np.asarray(inputs["conj2form"], np.int64)

    groups, cpf, cstart = _derive_structure(lit2conj, conj2form)
    inv, gk0 = _build_permutation(lit2conj, conj2form, groups, cpf, cstart)
    bias_zero = bool(np.all(bias == 0))
    nc = _build_program(groups, gk0, bias_zero)

    # weights: fp16, scaled, arranged [128p, 12plane, 4k, 896]
    wp = (weight[:, inv] * WS).astype(np.float16)
    mp = mask[:, inv].astype(np.float16)

    def arrange_w(a):  # [512, L] -> [128, 12, 4, 896]
        return np.ascontiguousarray(
            a.reshape(KT, 128, PLANES, CW).transpose(1, 2, 0, 3))

    w16 = arrange_w(wp)
    m16 = arrange_w(mp)

    # mu8: [128p, 2q, 2slot, 256]
    muT = np.ascontiguousarray(mu.T) * XS           # [512, 256]
    mu8 = np.ascontiguousarray(
        _q8(muT).reshape(2, 2, 128, F).transpose(2, 0, 1, 3)).reshape(128, -1)
    mun = np.ascontiguousarray(
        mu.reshape(2, 128, D).transpose(1, 0, 2)).reshape(128, -1)
    eye = np.eye(128, dtype=np.float16)

    in_maps = []
    for i in range(NCORES):
        xs = x[i * BS:(i + 1) * BS].T * XS          # [512, 1024]
        xs = np.clip(xs, -240.0, 240.0)
        xhi = _q8(xs)
        xlo = _q8(xs - xhi.astype(np.float32))
        # [128, 3(hi,lo,hi), 4k, 1024]
        xhl = np.empty((128, 3, KT, BS), ml_dtypes.float8_e4m3fn)
        xhl[:, 0] = xhi.reshape(KT, 128, BS).transpose(1, 0, 2)
        xhl[:, 1] = xlo.reshape(KT, 128, BS).transpose(1, 0, 2)
        xhl[:, 2] = xhl[:, 0]
        in_maps.append({
            "xhl": xhl.reshape(128, -1), "w16": w16, "m16": m16,
            "mu8": mu8, "mun": mun, "sig": sigma, "eye": eye,
        })
    return nc, in_maps


def kernel(x, weight, learnable_binary_mask, bias, mu, sigma,
           lit2conj, conj2form):
    inputs = {
        "x": x, "weight": weight,
        "learnable_binary_mask": learnable_binary_mask, "bias": bias,
        "mu": mu, "sigma": sigma, "lit2conj": lit2conj,
        "conj2form": conj2form,
    }
    nc, in_maps = prepare(inputs)
    res = bass_utils.run_bass_kernel_spmd(nc, in_maps,
                                          core_ids=list(range(NCORES)))
    out = np.concatenate(
        [res.results[i]["out"].reshape(128, NBT, F).transpose(1, 0, 2)
         .reshape(BS, F) for i in range(NCORES)], axis=0)
    return out.astype(np.float32)


# revision 24
# speedup vs baseline: 1.2159x; 1.1930x over previous
"""Trainium2 Bass kernel for nn_DNNF (segment_reduce DNF network).

Strategy: data-parallel over batch across 8 NeuronCores (1024 rows each).
Literal columns are host-permuted into 12 phase-planes of 896 so the AND
segment-sum becomes contiguous slab adds and the OR segment-sum contiguous
strided adds.  The literal GEMM runs in fp8 (e4m3) DoubleRow mode: plain
fp8 for the tanh-saturated depth-4/6 planes, hi+lo compensated fp8 for the
sensitive depth-2 planes (processed last so the PE-heavy burst overlaps the
scalar engine's conj/exp tail).  W and M stream in as fp16 and are fused
(multiply+fp8-cast) on Vector/GpSimd.  Scalar engine applies every tanh on
PSUM eviction.
"""
import numpy as np
import ml_dtypes

import concourse.bacc as bacc
import concourse.mybir as mybir
from concourse import bass_utils
from concourse.tile import TileContext

f32 = mybir.dt.float32
fp16 = mybir.dt.float16
fp8 = mybir.dt.float8e4
AX = mybir.AxisListType
ALU = mybir.AluOpType
ACTF = mybir.ActivationFunctionType
DR = mybir.MatmulPerfMode.DoubleRow

# problem shape (fixed by the harness)
B, D, L, C, F = 8192, 512, 10752, 2688, 256
NCORES = 8
BS = B // NCORES          # rows per core = 1024
NBT = BS // 128           # b-tiles per core = 8
KT = D // 128             # k-tiles = 4
CW = C // 3               # class width = 896 conj per depth-class
PLANES = L // CW          # 12 literal phase-planes
DEPTHS = (2, 4, 6)
PLANE_BASE = {2: 0, 4: 2, 6: 6}
CLS_OFF = {2: 0, 4: 1, 6: 2}
TEMPERATURE = 2.0
XS = 32.0                 # x fp8 scale
WS = 2048.0               # w fp16/fp8 scale
INV_SCALE = 1.0 / (XS * WS)

# plane processing order: depth-4, depth-6, then the PE-heavy compensated
# depth-2 planes last (overlaps scalar-engine conj/exp tail work)
ORDER = [2, 3, 4, 5, 6, 7, 8, 9, 10, 11, 0, 1]
CLASS_OF_PLANE = {0: 0, 1: 0, 2: 1, 3: 1, 4: 1, 5: 1,
                  6: 2, 7: 2, 8: 2, 9: 2, 10: 2, 11: 2}
FIRST_OF_CLASS = {0: 0, 1: 2, 2: 6}
LAST_OF_CLASS = {0: 1, 1: 5, 2: 11}

_PROGRAM_CACHE = {}


def _derive_structure(lit2conj, conj2form):
    """Validate the expected DNF structure and return group metadata."""
    depths = np.bincount(lit2conj, minlength=C)
    assert np.array_equal(depths, np.tile(np.array(DEPTHS), C // 3)), \
        "unexpected lit2conj structure"
    cpf = np.bincount(conj2form, minlength=F)
    groups = []          # (formula_start, n_formulas, cpf)
    i = 0
    while i < F:
        j = i
        while j < F and cpf[j] == cpf[i]:
            j += 1
        groups.append((i, j - i, int(cpf[i])))
        i = j
    for (_, nf, c_) in groups:
        assert c_ % 3 == 0, "conj-per-formula not divisible by 3"
    cstart = np.concatenate([[0], np.cumsum(cpf)[:-1]])
    assert np.all(cstart % 3 == 0), "formula conj ranges not 3-aligned"
    return groups, cpf, cstart


def _build_permutation(lit2conj, conj2form, groups, cpf, cstart):
    """Map each literal to its (plane, k) column; see baseline docstring."""
    conj_depth = np.bincount(lit2conj, minlength=C)
    gk0 = {}
    acc = 0
    for gi, (f0, nf, c_) in enumerate(groups):
        gk0[gi] = acc
        acc += nf * (c_ // 3)
    assert acc == CW
    group_of_formula = np.zeros(F, np.int64)
    for gi, (f0, nf, c_) in enumerate(groups):
        group_of_formula[f0:f0 + nf] = gi
    form_of_conj = np.asarray(conj2form, np.int64)
    g_of_conj = group_of_formula[form_of_conj]
    c3 = np.arange(C) // 3
    s3 = (cstart[form_of_conj] // 3).astype(np.int64)
    j_in_form = c3 - s3
    f_local = form_of_conj - np.asarray([groups[g][0] for g in g_of_conj])
    k_of_conj = (np.asarray([gk0[g] for g in g_of_conj])
                 + j_in_form * np.asarray([groups[g][1] for g in g_of_conj])
                 + f_local)
    first_lit = np.concatenate([[0], np.cumsum(conj_depth)[:-1]])
    lpos = np.arange(L) - first_lit[lit2conj]
    plane = np.asarray([PLANE_BASE[int(d)] for d in conj_depth[lit2conj]]) + lpos
    newcol = plane * CW + k_of_conj[lit2conj]
    assert len(np.unique(newcol)) == L
    inv = np.empty(L, np.int64)
    inv[newcol] = np.arange(L)
    return inv, gk0


def _build_program(groups, gk0, bias_zero):
    key = (tuple(groups), tuple(sorted(gk0.items())), bias_zero)
    if key in _PROGRAM_CACHE:
        return _PROGRAM_CACHE[key]
    assert bias_zero, "nonzero literal bias path not implemented"

    LN_T = float(np.log(TEMPERATURE))

    nc = bacc.Bacc("TRN2", target_bir_lowering=False, debug=False,
                   num_devices=NCORES)

    # dram inputs (host pre-arranged to sbuf layouts; see prepare())
    xhl_d = nc.dram_tensor("xhl", [128, KT * BS], fp8,
                           kind="ExternalInput").ap()
    x16_d = nc.dram_tensor("x16", [128, KT * BS], fp16,
                           kind="ExternalInput").ap()
    w16_d = nc.dram_tensor("w16", [128, PLANES, KT, CW], fp16,
                           kind="ExternalInput").ap()
    m16_d = nc.dram_tensor("m16", [128, PLANES, KT, CW], fp16,
                           kind="ExternalInput").ap()
    mu8_d = nc.dram_tensor("mu8", [128, 2 * 2 * F], fp8,
                           kind="ExternalInput").ap()
    mun_d = nc.dram_tensor("mun", [128, 2 * D], f32,
                           kind="ExternalInput").ap()
    sig_d = nc.dram_tensor("sig", [F], f32, kind="ExternalInput").ap()
    eye_d = nc.dram_tensor("eye", [128, 128], fp16, kind="ExternalInput").ap()
    out_d = nc.dram_tensor("out", [128, NBT * F], f32,
                           kind="ExternalOutput").ap()
    scr_d = nc.dram_tensor("bscr", [F], fp16, kind="Internal").ap()
    scr2_d = nc.dram_tensor("ascr", [F], f32, kind="Internal").ap()

    with TileContext(nc) as tc:
        with tc.tile_pool(name="cst", bufs=1) as cst, \
             tc.tile_pool(name="wst", bufs=2) as wpool, \
             tc.tile_pool(name="mst", bufs=2) as mpool, \
             tc.tile_pool(name="cur", bufs=2) as curp, \
             tc.tile_pool(name="wm16p", bufs=1) as wm16p, \
             tc.tile_pool(name="ps", bufs=4, space="PSUM") as psp:

            bias_cols = {}

            def bias_col(val):
                v = float(val)
                if v not in bias_cols:
                    t = cst.tile([128, 1], f32, tag=f"bc{len(bias_cols)}")
                    nc.vector.memset(t[:], v)
                    bias_cols[v] = t
                return bias_cols[v][:]

            # ---------- resident tiles ----------
            xhl = cst.tile([128, KT, BS], fp8, tag="xhl")

            def load_xhl():
                nc.sync.dma_start(
                    xhl[:].rearrange("p a b -> p (a b)"), xhl_d[:, :])
            x16 = cst.tile([128, KT, BS], fp16, tag="x16")
            nc.sync.dma_start(
                x16[:].rearrange("p a b -> p (a b)"), x16_d[:, :])
            wm8 = cst.tile([128, KT, PLANES - 2, CW], fp8, tag="wm8")
            wmd2 = cst.tile([128, KT, 2 * CW], fp16, tag="wmd2")
            conj0 = cst.tile([128, NBT, CW], fp16, tag="conj0")
            conj1 = cst.tile([128, NBT, CW], fp16, tag="conj1")
            conj2 = cst.tile([128, NBT, CW], fp16, tag="conj2")
            conj_of = {0: conj0, 1: conj1, 2: conj2}
            por = cst.tile([128, 3, NBT, F], fp16, tag="por")
            form = cst.tile([128, NBT, F], f32, tag="form")
            z_all = cst.tile([128, NBT, F], fp16, tag="z_all")
            e_t = z_all  # Exp applied twice in place; z dead after
            s_t = cst.tile([128, NBT], f32, tag="s_t")
            r_t = cst.tile([128, NBT], f32, tag="r_t")
            mu8_t = cst.tile([128, 2, 2, F], fp8, tag="mu8")
            nc.sync.dma_start(
                mu8_t[:].rearrange("p a b c -> p (a b c)"), mu8_d[:, :])
            eye_t = cst.tile([128, 128], fp16, tag="eye")
            nc.sync.dma_start(eye_t[:], eye_d[:])

            # ---------- loc constants: m2, beta row, a' row ----------
            mun_t = cst.tile([128, 2, D], f32, tag="mun")
            nc.sync.dma_start(
                mun_t[:].rearrange("p a b -> p (a b)"), mun_d[:, :])
            nc.gpsimd.tensor_mul(mun_t[:], mun_t[:], mun_t[:])
            m2c = cst.tile([128, 2], f32, tag="m2c")
            nc.vector.reduce_sum(m2c[:], mun_t[:], axis=AX.X)
            sigc = cst.tile([128, 2], f32, tag="sigc")
            nc.sync.dma_start(sigc[:], sig_d.rearrange("(a p) -> p a", p=128))
            s2c = cst.tile([128, 2], f32, tag="s2c")
            nc.gpsimd.tensor_mul(s2c[:], sigc[:], sigc[:])
            # beta = -0.5*m2 + ln(T)*sigma^2 ; store 32*beta as fp16 row
            bc1 = cst.tile([128, 2], f32, tag="bc1")
            nc.gpsimd.tensor_scalar_mul(bc1[:], m2c[:], -0.5 * XS)
            bc2 = cst.tile([128, 2], f32, tag="bc2")
            nc.gpsimd.tensor_scalar_mul(bc2[:], s2c[:], LN_T * XS)
            bcol = cst.tile([128, 2], f32, tag="bcol")
            nc.vector.tensor_add(bcol[:], bc1[:], bc2[:])
            nc.sync.dma_start(scr_d.rearrange("(a p) -> p a", p=128), bcol[:])
            brow = cst.tile([1, F], f32, tag="brow")
            nc.sync.dma_start(brow[:], scr_d[None, :])
            brow16 = cst.tile([1, F], fp16, tag="brow16")
            nc.vector.tensor_copy(brow16[:], brow[:])
            # a' = 1/(sigma^2 * XS^2) as fp16 row (z scale absorbed)
            acol = cst.tile([128, 2], f32, tag="acol")
            nc.vector.reciprocal(acol[:], s2c[:])
            acol2 = cst.tile([128, 2], f32, tag="acol2")
            nc.gpsimd.tensor_scalar_mul(acol2[:], acol[:], 1.0 / (XS * XS))
            nc.sync.dma_start(scr2_d.rearrange("(a p) -> p a", p=128),
                              acol2[:])
            arow = cst.tile([1, F], f32, tag="arow")
            nc.sync.dma_start(arow[:], scr2_d[None, :])
            arow16 = cst.tile([1, F], fp16, tag="arow16")
            nc.vector.tensor_copy(arow16[:], arow[:])
            ones1 = cst.tile([1, 128], fp16, tag="ones1")
            nc.vector.memset(ones1[:], 1.0)
            o32 = cst.tile([1, 128], fp16, tag="o32")
            nc.vector.memset(o32[:], XS)

            # a' broadcast to [128, F] via rank-1 matmul
            ps_a = psp.tile([128, 1024], f32, tag="ps")
            nc.tensor.matmul(ps_a[:, 0:F], ones1[:], arow16[:],
                             start=True, stop=True)
            a_bc = cst.tile([128, F], f32, tag="a_bc")
            nc.vector.tensor_copy(a_bc[:], ps_a[:, 0:F])

            # ---------- helpers ----------
            def naive_lhsT(q, b):
                # slots = k-tiles (2q, 2q+1) of x-hi
                return xhl[:, 2 * q:2 * q + 2, b * 128:(b + 1) * 128]

            def do_plane(p):
                ci = CLASS_OF_PLANE[p]
                d2 = (ci == 0)
                wt = wpool.tile([128, KT, CW], fp16, tag="w")
                nc.sync.dma_start(
                    wt[:].rearrange("p a b -> p (a b)"),
                    w16_d[:, p].rearrange("p a b -> p (a b)"))
                mt = mpool.tile([128, KT, CW], fp16, tag="m")
                nc.sync.dma_start(
                    mt[:].rearrange("p a b -> p (a b)"),
                    m16_d[:, p].rearrange("p a b -> p (a b)"))
                if post_dma is not None:
                    post_dma()
                if d2:
                    # depth-2 planes run in fp16: product only, no fp8 cast
                    dst = wmd2[:, :, p * CW:(p + 1) * CW]
                    nc.vector.tensor_mul(dst, wt[:], mt[:])
                else:
                    dst = wm8[:, :, p - 2, :]
                    nc.vector.tensor_mul(dst[:, 0:2], wt[:, 0:2], mt[:, 0:2])
                    nc.gpsimd.tensor_mul(dst[:, 2:4], wt[:, 2:4], mt[:, 2:4])

                first = (p == FIRST_OF_CLASS[ci])
                target = conj_of[ci] if first else curp.tile(
                    [128, NBT, CW], fp16, tag="cur")
                for b in range(NBT):
                    ps = psp.tile([128, 1024], f32, tag="ps")
                    bsl = slice(b * 128, (b + 1) * 128)
                    if d2:
                        for co, cw_ in ((0, 512), (512, 384)):
                            for k in range(KT):
                                nc.tensor.matmul(
                                    ps[:, co:co + cw_], x16[:, k, bsl],
                                    wmd2[:, k, p * CW + co:p * CW + co + cw_],
                                    start=(k == 0), stop=(k == KT - 1))
                    else:
                        for co, cw_ in ((0, 512), (512, 384)):
                            for q in range(2):
                                nc.tensor.matmul(
                                    ps[:, co:co + cw_], naive_lhsT(q, b),
                                    wm8[:, 2 * q:2 * q + 2, p - 2,
                                        co:co + cw_],
                                    start=(q == 0), stop=(q == 1),
                                    perf_mode=DR)
                    nc.scalar.activation(
                        target[:, b, :], ps[:, 0:CW], ACTF.Tanh,
                        scale=(1.0 / WS) if d2 else INV_SCALE)
                if not first:
                    nc.vector.tensor_add(conj_of[ci][:], conj_of[ci][:],
                                         target[:])
                if p == LAST_OF_CLASS[ci]:
                    close_class(ci)

            def close_class(ci, bs_):
                d = DEPTHS[ci]
                cj = conj_of[ci]
                nc.scalar.activation(cj[:, bs_], cj[:, bs_], ACTF.Tanh,
                                     bias=bias_col(1.5 - d))
                # OR partial trees into por[ci]
                for gi, (f0, nf, cpf_g) in enumerate(groups):
                    mg = cpf_g // 3
                    k0 = gk0[gi]
                    dst = por[:, ci, bs_, f0:f0 + nf]
                    nc.vector.tensor_add(dst, cj[:, bs_, k0:k0 + nf],
                                         cj[:, bs_, k0 + nf:k0 + 2 * nf])
                    for j in range(2, mg):
                        nc.vector.tensor_add(
                            dst, dst,
                            cj[:, bs_, k0 + j * nf:k0 + (j + 1) * nf])

            def emit_loc_b(b):
                # z = a' * (G_raw + beta_raw - 0.5*||xs||^2) ; raw = XS^2-scaled
                if True:
                    bsl = slice(b * 128, (b + 1) * 128)
                    ps_g = psp.tile([128, 1024], f32, tag="ps")
                    for q in range(2):
                        nc.tensor.matmul(ps_g[:, 0:128],
                                         naive_lhsT(q, b), naive_lhsT(q, b),
                                         start=(q == 0), stop=(q == 1),
                                         perf_mode=DR)
                    gd = cst.tile([128, 128], f32, tag=f"gd{b % 2}")
                    gdv = gd[:]
                    nc.vector.tensor_mul(gdv, ps_g[:, 0:128], eye_t[:])
                    sqc = cst.tile([128, 1], f32, tag=f"sq{b}")
                    nc.vector.reduce_sum(sqc[:], gdv, axis=AX.XYZW)
                    sqh = cst.tile([128, 1], f32, tag=f"sqh{b}")
                    nc.vector.tensor_scalar_mul(sqh[:], sqc[:], 0.5)

                    ps_z = psp.tile([128, 1024], f32, tag="ps")
                    for q in range(2):
                        nc.tensor.matmul(ps_z[:, 0:F], naive_lhsT(q, b),
                                         mu8_t[:, q], start=(q == 0),
                                         stop=False, perf_mode=DR)
                    nc.tensor.matmul(ps_z[:, 0:F], o32[:], brow16[:],
                                     start=False, stop=True,
                                     skip_group_check=True)
                    nc.vector.scalar_tensor_tensor(
                        z_all[:, b, :], ps_z[:, 0:F], sqh[:],
                        consts["a_bc"][:],
                        op0=ALU.subtract, op1=ALU.mult)

            def finish():
                # z -> T*e^w -> softmax pieces
                nc.scalar.activation(z_all[:], z_all[:], ACTF.Exp, bias=0.0)
                nc.scalar.activation(e_t[:], z_all[:], ACTF.Exp, bias=0.0)
                nc.vector.reduce_sum(s_t[:], e_t[:], axis=AX.X)
                nc.vector.reciprocal(r_t[:], s_t[:])
                # form = sum of class partials (f32), tanh with per-group bias
                nc.vector.tensor_add(form[:], por[:, 0], por[:, 1])
                nc.vector.tensor_add(form[:], form[:], por[:, 2])
                for gi, (f0, nf, cpf_g) in enumerate(groups):
                    fv = form[:, :, f0:f0 + nf]
                    nc.scalar.activation(fv, fv, ACTF.Tanh,
                                         bias=bias_col(cpf_g - 1.5))
                nc.vector.tensor_mul(form[:], form[:], e_t[:])
                for b in range(NBT):
                    nc.vector.tensor_scalar_mul(form[:, b, :], form[:, b, :],
                                                r_t[:, b:b + 1])
                nc.sync.dma_start(
                    out_d[:, :], form[:].rearrange("p a b -> p (a b)"))

            # ---------- schedule ----------
            for i, p in enumerate(ORDER):
                do_plane(p)
                if i == 1:
                    do_loc()
            finish()

    nc.compile()
    _PROGRAM_CACHE[key] = nc
    return nc


def _q8(a):
    return np.clip(a, -240.0, 240.0).astype(ml_dtypes.float8_e4m3fn)


def prepare(inputs):
    """Host-side shard/layout prep. Returns (nc, in_maps)."""
    x = np.asarray(inputs["x"], np.float32)
    weight = np.asarray(inputs["weight"], np.float32)
    mask = np.asarray(inputs["learnable_binary_mask"], np.float32)
    bias = np.asarray(inputs["bias"], np.float32)
    mu = np.asarray(inputs["mu"], np.float32)
    sigma = np.asarray(inputs["sigma"], np.float32)
    lit2conj = np.asarray(inputs["lit2conj"], np.int64)
    conj2form = np.asarray(inputs["conj2form"], np.int64)

    groups, cpf, cstart = _derive_structure(lit2conj, conj2form)
    inv, gk0 = _build_permutation(lit2conj, conj2form, groups, cpf, cstart)
    bias_zero = bool(np.all(bias == 0))
    nc = _build_program(groups, gk0, bias_zero)

    # weights: fp16, scaled, arranged [128p, 12plane, 4k, 896]
    wp = (weight[:, inv] * WS).astype(np.float16)
    mp = mask[:, inv].astype(np.float16)

    def arrange_w(a):  # [512, L] -> [128, 12, 4, 896]
        return np.ascontiguousarray(
            a.reshape(KT, 128, PLANES, CW).transpose(1, 2, 0, 3))

    w16 = arrange_w(wp)
    m16 = arrange_w(mp)

    # mu8: [128p, 2q, 2slot, 256]
    muT = np.ascontiguousarray(mu.T) * XS           # [512, 256]
    mu8 = np.ascontiguousarray(
        _q8(muT).reshape(2, 2, 128, F).transpose(2, 0, 1, 3)).reshape(128, -1)
    mun = np.ascontiguousarray(
        mu.reshape(2, 128, D).transpose(1, 0, 2)).reshape(128, -1)
    eye = np.eye(128, dtype=np.float16)

    in_maps = []
    for i in range(NCORES):
        xT = x[i * BS:(i + 1) * BS].T               # [512, 1024]
        xs = np.clip(xT * XS, -240.0, 240.0)
        xhl = np.ascontiguousarray(
            _q8(xs).reshape(KT, 128, BS).transpose(1, 0, 2)).reshape(128, -1)
        x16 = np.ascontiguousarray(
            xT.astype(np.float16).reshape(KT, 128, BS)
            .transpose(1, 0, 2)).reshape(128, -1)
        in_maps.append({
            "xhl": xhl, "x16": x16, "w16": w16, "m16": m16,
            "mu8": mu8, "mun": mun, "sig": sigma, "eye": eye,
        })
    return nc, in_maps


def kernel(x, weight, learnable_binary_mask, bias, mu, sigma,
           lit2conj, conj2form):
    inputs = {
        "x": x, "weight": weight,
        "learnable_binary_mask": learnable_binary_mask, "bias": bias,
        "mu": mu, "sigma": sigma, "lit2conj": lit2conj,
        "conj2form": conj2form,
    }
    nc, in_maps = prepare(inputs)
    res = bass_utils.run_bass_kernel_spmd(nc, in_maps,
                                          core_ids=list(range(NCORES)))
    out = np.concatenate(
        [res.results[i]["out"].reshape(128, NBT, F).transpose(1, 0, 2)
         .reshape(BS, F) for i in range(NCORES)], axis=0)
    return out.astype(np.float32)


# revision 26
# speedup vs baseline: 1.4145x; 1.1633x over previous
"""Trainium2 Bass kernel for nn_DNNF (segment_reduce DNF network).

Strategy: data-parallel over batch across 8 NeuronCores (1024 rows each).
Literal columns are host-permuted into 12 phase-planes of 896 so the AND
segment-sum becomes contiguous slab adds and the OR segment-sum contiguous
strided adds.  The literal GEMM runs in fp8 (e4m3) DoubleRow mode: plain
fp8 for the tanh-saturated depth-4/6 planes, hi+lo compensated fp8 for the
sensitive depth-2 planes (processed last so the PE-heavy burst overlaps the
scalar engine's conj/exp tail).  W and M stream in as fp16 and are fused
(multiply+fp8-cast) on Vector/GpSimd.  Scalar engine applies every tanh on
PSUM eviction.
"""
import numpy as np
import ml_dtypes

import concourse.bacc as bacc
import concourse.mybir as mybir
from concourse import bass_utils
from concourse.tile import TileContext

f32 = mybir.dt.float32
fp16 = mybir.dt.float16
fp8 = mybir.dt.float8e4
AX = mybir.AxisListType
ALU = mybir.AluOpType
ACTF = mybir.ActivationFunctionType
DR = mybir.MatmulPerfMode.DoubleRow

# problem shape (fixed by the harness)
B, D, L, C, F = 8192, 512, 10752, 2688, 256
NCORES = 8
BS = B // NCORES          # rows per core = 1024
NBT = BS // 128           # b-tiles per core = 8
KT = D // 128             # k-tiles = 4
CW = C // 3               # class width = 896 conj per depth-class
PLANES = L // CW          # 12 literal phase-planes
DEPTHS = (2, 4, 6)
PLANE_BASE = {2: 0, 4: 2, 6: 6}
CLS_OFF = {2: 0, 4: 1, 6: 2}
TEMPERATURE = 2.0
XS = 32.0                 # x fp8 scale
WS = 2048.0               # w fp16/fp8 scale
INV_SCALE = 1.0 / (XS * WS)

# plane processing order: depth-4, depth-6, then the PE-heavy compensated
# depth-2 planes last (overlaps scalar-engine conj/exp tail work)
ORDER = [2, 3, 4, 5, 6, 7, 8, 9, 10, 11, 0, 1]
CLASS_OF_PLANE = {0: 0, 1: 0, 2: 1, 3: 1, 4: 1, 5: 1,
                  6: 2, 7: 2, 8: 2, 9: 2, 10: 2, 11: 2}
FIRST_OF_CLASS = {0: 0, 1: 2, 2: 6}
LAST_OF_CLASS = {0: 1, 1: 5, 2: 11}

_PROGRAM_CACHE = {}


def _derive_structure(lit2conj, conj2form):
    """Validate the expected DNF structure and return group metadata."""
    depths = np.bincount(lit2conj, minlength=C)
    assert np.array_equal(depths, np.tile(np.array(DEPTHS), C // 3)), \
        "unexpected lit2conj structure"
    cpf = np.bincount(conj2form, minlength=F)
    groups = []          # (formula_start, n_formulas, cpf)
    i = 0
    while i < F:
        j = i
        while j < F and cpf[j] == cpf[i]:
            j += 1
        groups.append((i, j - i, int(cpf[i])))
        i = j
    for (_, nf, c_) in groups:
        assert c_ % 3 == 0, "conj-per-formula not divisible by 3"
    cstart = np.concatenate([[0], np.cumsum(cpf)[:-1]])
    assert np.all(cstart % 3 == 0), "formula conj ranges not 3-aligned"
    return groups, cpf, cstart


def _build_permutation(lit2conj, conj2form, groups, cpf, cstart):
    """Map each literal to its (plane, k) column; see baseline docstring."""
    conj_depth = np.bincount(lit2conj, minlength=C)
    gk0 = {}
    acc = 0
    for gi, (f0, nf, c_) in enumerate(groups):
        gk0[gi] = acc
        acc += nf * (c_ // 3)
    assert acc == CW
    group_of_formula = np.zeros(F, np.int64)
    for gi, (f0, nf, c_) in enumerate(groups):
        group_of_formula[f0:f0 + nf] = gi
    form_of_conj = np.asarray(conj2form, np.int64)
    g_of_conj = group_of_formula[form_of_conj]
    c3 = np.arange(C) // 3
    s3 = (cstart[form_of_conj] // 3).astype(np.int64)
    j_in_form = c3 - s3
    f_local = form_of_conj - np.asarray([groups[g][0] for g in g_of_conj])
    k_of_conj = (np.asarray([gk0[g] for g in g_of_conj])
                 + j_in_form * np.asarray([groups[g][1] for g in g_of_conj])
                 + f_local)
    first_lit = np.concatenate([[0], np.cumsum(conj_depth)[:-1]])
    lpos = np.arange(L) - first_lit[lit2conj]
    plane = np.asarray([PLANE_BASE[int(d)] for d in conj_depth[lit2conj]]) + lpos
    newcol = plane * CW + k_of_conj[lit2conj]
    assert len(np.unique(newcol)) == L
    inv = np.empty(L, np.int64)
    inv[newcol] = np.arange(L)
    return inv, gk0


def _build_program(groups, gk0, bias_zero):
    key = (tuple(groups), tuple(sorted(gk0.items())), bias_zero)
    if key in _PROGRAM_CACHE:
        return _PROGRAM_CACHE[key]
    assert bias_zero, "nonzero literal bias path not implemented"

    LN_T = float(np.log(TEMPERATURE))

    nc = bacc.Bacc("TRN2", target_bir_lowering=False, debug=False,
                   num_devices=NCORES)

    # dram inputs (host pre-arranged to sbuf layouts; see prepare())
    xhl_d = nc.dram_tensor("xhl", [128, KT * BS], fp8,
                           kind="ExternalInput").ap()
    x16_d = nc.dram_tensor("x16", [128, KT * BS], fp16,
                           kind="ExternalInput").ap()
    w16_d = nc.dram_tensor("w16", [128, PLANES, KT, CW], fp16,
                           kind="ExternalInput").ap()
    m16_d = nc.dram_tensor("m16", [128, PLANES, KT, CW], fp16,
                           kind="ExternalInput").ap()
    mu8_d = nc.dram_tensor("mu8", [128, 2 * 2 * F], fp8,
                           kind="ExternalInput").ap()
    mun_d = nc.dram_tensor("mun", [128, 2 * D], f32,
                           kind="ExternalInput").ap()
    sig_d = nc.dram_tensor("sig", [F], f32, kind="ExternalInput").ap()
    eye_d = nc.dram_tensor("eye", [128, 128], fp16, kind="ExternalInput").ap()
    out_d = nc.dram_tensor("out", [128, NBT * F], f32,
                           kind="ExternalOutput").ap()
    scr_d = nc.dram_tensor("bscr", [F], fp16, kind="Internal").ap()
    scr2_d = nc.dram_tensor("ascr", [F], f32, kind="Internal").ap()

    with TileContext(nc) as tc:
        with tc.tile_pool(name="cst", bufs=1) as cst, \
             tc.tile_pool(name="wst", bufs=2) as wpool, \
             tc.tile_pool(name="mst", bufs=2) as mpool, \
             tc.tile_pool(name="cur", bufs=2) as curp, \
             tc.tile_pool(name="wm16p", bufs=1) as wm16p, \
             tc.tile_pool(name="ps", bufs=4, space="PSUM") as psp:

            bias_cols = {}

            def bias_col(val):
                v = float(val)
                if v not in bias_cols:
                    t = cst.tile([128, 1], f32, tag=f"bc{len(bias_cols)}")
                    nc.vector.memset(t[:], v)
                    bias_cols[v] = t
                return bias_cols[v][:]

            # ---------- resident tiles ----------
            xhl = cst.tile([128, KT, BS], fp8, tag="xhl")

            def load_xhl():
                nc.sync.dma_start(
                    xhl[:].rearrange("p a b -> p (a b)"), xhl_d[:, :])
            x16 = cst.tile([128, KT, BS], fp16, tag="x16")
            nc.sync.dma_start(
                x16[:].rearrange("p a b -> p (a b)"), x16_d[:, :])
            wm8 = cst.tile([128, KT, PLANES - 2, CW], fp8, tag="wm8")
            wmd2 = cst.tile([128, KT, 2 * CW], fp16, tag="wmd2")
            conj0 = cst.tile([128, NBT, CW], fp16, tag="conj0")
            conj1 = cst.tile([128, NBT, CW], fp16, tag="conj1")
            conj2 = cst.tile([128, NBT, CW], fp16, tag="conj2")
            conj_of = {0: conj0, 1: conj1, 2: conj2}
            por = cst.tile([128, 3, NBT, F], fp16, tag="por")
            form = cst.tile([128, NBT, F], f32, tag="form")
            z_all = cst.tile([128, NBT, F], fp16, tag="z_all")
            e_t = z_all  # Exp applied twice in place; z dead after
            s_t = cst.tile([128, NBT], f32, tag="s_t")
            r_t = cst.tile([128, NBT], f32, tag="r_t")
            mu8_t = cst.tile([128, 2, 2, F], fp8, tag="mu8")
            nc.sync.dma_start(
                mu8_t[:].rearrange("p a b c -> p (a b c)"), mu8_d[:, :])
            eye_t = cst.tile([128, 128], fp16, tag="eye")
            nc.sync.dma_start(eye_t[:], eye_d[:])

            # ---------- loc constants: m2, beta row, a' row ----------
            mun_t = cst.tile([128, 2, D], f32, tag="mun")
            nc.sync.dma_start(
                mun_t[:].rearrange("p a b -> p (a b)"), mun_d[:, :])
            nc.gpsimd.tensor_mul(mun_t[:], mun_t[:], mun_t[:])
            m2c = cst.tile([128, 2], f32, tag="m2c")
            nc.vector.reduce_sum(m2c[:], mun_t[:], axis=AX.X)
            sigc = cst.tile([128, 2], f32, tag="sigc")
            nc.sync.dma_start(sigc[:], sig_d.rearrange("(a p) -> p a", p=128))
            s2c = cst.tile([128, 2], f32, tag="s2c")
            nc.gpsimd.tensor_mul(s2c[:], sigc[:], sigc[:])
            # beta = -0.5*m2 + ln(T)*sigma^2 ; store 32*beta as fp16 row
            bc1 = cst.tile([128, 2], f32, tag="bc1")
            nc.gpsimd.tensor_scalar_mul(bc1[:], m2c[:], -0.5 * XS)
            bc2 = cst.tile([128, 2], f32, tag="bc2")
            nc.gpsimd.tensor_scalar_mul(bc2[:], s2c[:], LN_T * XS)
            bcol = cst.tile([128, 2], f32, tag="bcol")
            nc.vector.tensor_add(bcol[:], bc1[:], bc2[:])
            nc.sync.dma_start(scr_d.rearrange("(a p) -> p a", p=128), bcol[:])
            brow = cst.tile([1, F], f32, tag="brow")
            nc.sync.dma_start(brow[:], scr_d[None, :])
            brow16 = cst.tile([1, F], fp16, tag="brow16")
            nc.vector.tensor_copy(brow16[:], brow[:])
            # a' = 1/(sigma^2 * XS^2) as fp16 row (z scale absorbed)
            acol = cst.tile([128, 2], f32, tag="acol")
            nc.vector.reciprocal(acol[:], s2c[:])
            acol2 = cst.tile([128, 2], f32, tag="acol2")
            nc.gpsimd.tensor_scalar_mul(acol2[:], acol[:], 1.0 / (XS * XS))
            nc.sync.dma_start(scr2_d.rearrange("(a p) -> p a", p=128),
                              acol2[:])
            arow = cst.tile([1, F], f32, tag="arow")
            nc.sync.dma_start(arow[:], scr2_d[None, :])
            arow16 = cst.tile([1, F], fp16, tag="arow16")
            nc.vector.tensor_copy(arow16[:], arow[:])
            ones1 = cst.tile([1, 128], fp16, tag="ones1")
            nc.vector.memset(ones1[:], 1.0)
            o32 = cst.tile([1, 128], fp16, tag="o32")
            nc.vector.memset(o32[:], XS)

            # a' broadcast to [128, F] via rank-1 matmul
            ps_a = psp.tile([128, 1024], f32, tag="ps")
            nc.tensor.matmul(ps_a[:, 0:F], ones1[:], arow16[:],
                             start=True, stop=True)
            a_bc = cst.tile([128, F], f32, tag="a_bc")
            nc.vector.tensor_copy(a_bc[:], ps_a[:, 0:F])

            # ---------- helpers ----------
            def naive_lhsT(q, b):
                # slots = k-tiles (2q, 2q+1) of x-hi
                return xhl[:, 2 * q:2 * q + 2, b * 128:(b + 1) * 128]

            def do_plane(p):
                ci = CLASS_OF_PLANE[p]
                d2 = (ci == 0)
                wt = wpool.tile([128, KT, CW], fp16, tag="w")
                nc.sync.dma_start(
                    wt[:].rearrange("p a b -> p (a b)"),
                    w16_d[:, p].rearrange("p a b -> p (a b)"))
                mt = mpool.tile([128, KT, CW], fp16, tag="m")
                nc.sync.dma_start(
                    mt[:].rearrange("p a b -> p (a b)"),
                    m16_d[:, p].rearrange("p a b -> p (a b)"))
                if post_dma is not None:
                    post_dma()
                if d2:
                    # depth-2 planes run in fp16: product only, no fp8 cast
                    dst = wmd2[:, :, p * CW:(p + 1) * CW]
                    nc.vector.tensor_mul(dst, wt[:], mt[:])
                else:
                    dst = wm8[:, :, p - 2, :]
                    nc.vector.tensor_mul(dst[:, 0:2], wt[:, 0:2], mt[:, 0:2])
                    nc.gpsimd.tensor_mul(dst[:, 2:4], wt[:, 2:4], mt[:, 2:4])

                first = (p == FIRST_OF_CLASS[ci])
                target = conj_of[ci] if first else curp.tile(
                    [128, NBT, CW], fp16, tag="cur")
                for b in range(NBT):
                    ps = psp.tile([128, 1024], f32, tag="ps")
                    bsl = slice(b * 128, (b + 1) * 128)
                    if d2:
                        for co, cw_ in ((0, 512), (512, 384)):
                            for k in range(KT):
                                nc.tensor.matmul(
                                    ps[:, co:co + cw_], x16[:, k, bsl],
                                    wmd2[:, k, p * CW + co:p * CW + co + cw_],
                                    start=(k == 0), stop=(k == KT - 1))
                    else:
                        for co, cw_ in ((0, 512), (512, 384)):
                            for q in range(2):
                                nc.tensor.matmul(
                                    ps[:, co:co + cw_], naive_lhsT(q, b),
                                    wm8[:, 2 * q:2 * q + 2, p - 2,
                                        co:co + cw_],
                                    start=(q == 0), stop=(q == 1),
                                    perf_mode=DR)
                    nc.scalar.activation(
                        target[:, b, :], ps[:, 0:CW], ACTF.Tanh,
                        scale=(1.0 / WS) if d2 else INV_SCALE)
                if not first:
                    nc.vector.tensor_add(conj_of[ci][:], conj_of[ci][:],
                                         target[:])
                if p == LAST_OF_CLASS[ci]:
                    close_class(ci)

            def close_class(ci, bs_):
                d = DEPTHS[ci]
                cj = conj_of[ci]
                nc.scalar.activation(cj[:, bs_], cj[:, bs_], ACTF.Tanh,
                                     bias=bias_col(1.5 - d))
                # OR partial trees into por[ci]
                for gi, (f0, nf, cpf_g) in enumerate(groups):
                    mg = cpf_g // 3
                    k0 = gk0[gi]
                    dst = por[:, ci, bs_, f0:f0 + nf]
                    nc.vector.tensor_add(dst, cj[:, bs_, k0:k0 + nf],
                                         cj[:, bs_, k0 + nf:k0 + 2 * nf])
                    for j in range(2, mg):
                        nc.vector.tensor_add(
                            dst, dst,
                            cj[:, bs_, k0 + j * nf:k0 + (j + 1) * nf])

            def emit_loc_b(b):
                # z = a' * (G_raw + beta_raw - 0.5*||xs||^2) ; raw = XS^2-scaled
                if True:
                    bsl = slice(b * 128, (b + 1) * 128)
                    ps_g = psp.tile([128, 1024], f32, tag="ps")
                    for q in range(2):
                        nc.tensor.matmul(ps_g[:, 0:128],
                                         naive_lhsT(q, b), naive_lhsT(q, b),
                                         start=(q == 0), stop=(q == 1),
                                         perf_mode=DR)
                    gd = cst.tile([128, 128], f32, tag=f"gd{b % 2}")
                    gdv = gd[:]
                    nc.vector.tensor_mul(gdv, ps_g[:, 0:128], eye_t[:])
                    sqc = cst.tile([128, 1], f32, tag=f"sq{b}")
                    nc.vector.reduce_sum(sqc[:], gdv, axis=AX.XYZW)
                    sqh = cst.tile([128, 1], f32, tag=f"sqh{b}")
                    nc.vector.tensor_scalar_mul(sqh[:], sqc[:], 0.5)

                    ps_z = psp.tile([128, 1024], f32, tag="ps")
                    for q in range(2):
                        nc.tensor.matmul(ps_z[:, 0:F], naive_lhsT(q, b),
                                         mu8_t[:, q], start=(q == 0),
                                         stop=False, perf_mode=DR)
                    nc.tensor.matmul(ps_z[:, 0:F], o32[:], brow16[:],
                                     start=False, stop=True,
                                     skip_group_check=True)
                    nc.vector.scalar_tensor_tensor(
                        z_all[:, b, :], ps_z[:, 0:F], sqh[:],
                        consts["a_bc"][:],
                        op0=ALU.subtract, op1=ALU.mult)

            def finish():
                # z -> T*e^w -> softmax pieces
                nc.scalar.activation(z_all[:], z_all[:], ACTF.Exp, bias=0.0)
                nc.scalar.activation(e_t[:], z_all[:], ACTF.Exp, bias=0.0)
                nc.vector.reduce_sum(s_t[:], e_t[:], axis=AX.X)
                nc.vector.reciprocal(r_t[:], s_t[:])
                # form = sum of class partials (f32), tanh with per-group bias
                nc.vector.tensor_add(form[:], por[:, 0], por[:, 1])
                nc.vector.tensor_add(form[:], form[:], por[:, 2])
                for gi, (f0, nf, cpf_g) in enumerate(groups):
                    fv = form[:, :, f0:f0 + nf]
                    nc.scalar.activation(fv, fv, ACTF.Tanh,
                                         bias=bias_col(cpf_g - 1.5))
                nc.vector.tensor_mul(form[:], form[:], e_t[:])
                for b in range(NBT):
                    nc.vector.tensor_scalar_mul(form[:, b, :], form[:, b, :],
                                                r_t[:, b:b + 1])
                nc.sync.dma_start(
                    out_d[:, :], form[:].rearrange("p a b -> p (a b)"))

            # ---------- schedule ----------
            for i, p in enumerate(ORDER):
                do_plane(p)
                if i == 1:
                    do_loc()
            finish()

    nc.compile()
    _PROGRAM_CACHE[key] = nc
    return nc


def _q8(a):
    return np.clip(a, -240.0, 240.0).astype(ml_dtypes.float8_e4m3fn)


def prepare(inputs):
    """Host-side shard/layout prep. Returns (nc, in_maps)."""
    x = np.asarray(inputs["x"], np.float32)
    weight = np.asarray(inputs["weight"], np.float32)
    mask = np.asarray(inputs["learnable_binary_mask"], np.float32)
    bias = np.asarray(inputs["bias"], np.float32)
    mu = np.asarray(inputs["mu"], np.float32)
    sigma = np.asarray(inputs["sigma"], np.float32)
    lit2conj = np.asarray(inputs["lit2conj"], np.int64)
    conj2form = np.asarray(inputs["conj2form"], np.int64)

    groups, cpf, cstart = _derive_structure(lit2conj, conj2form)
    inv, gk0 = _build_permutation(lit2conj, conj2form, groups, cpf, cstart)
    bias_zero = bool(np.all(bias == 0))
    nc = _build_program(groups, gk0, bias_zero)

    # weights: fp16, scaled, arranged [128p, 12plane, 4k, 896]
    wp = (weight[:, inv] * WS).astype(np.float16)
    mp = mask[:, inv].astype(np.float16)

    def arrange_w(a):  # [512, L] -> [128, 12, 4, 896]
        return np.ascontiguousarray(
            a.reshape(KT, 128, PLANES, CW).transpose(1, 2, 0, 3))

    w16 = arrange_w(wp)
    m16 = arrange_w(mp)

    # mu8: [128p, 2q, 2slot, 256]
    muT = np.ascontiguousarray(mu.T) * XS           # [512, 256]
    mu8 = np.ascontiguousarray(
        _q8(muT).reshape(2, 2, 128, F).transpose(2, 0, 1, 3)).reshape(128, -1)
    mun = np.ascontiguousarray(
        mu.reshape(2, 128, D).transpose(1, 0, 2)).reshape(128, -1)
    eye = np.eye(128, dtype=np.float16)

    in_maps = []
    for i in range(NCORES):
        xT = x[i * BS:(i + 1) * BS].T               # [512, 1024]
        xs = np.clip(xT * XS, -240.0, 240.0)
        xhl = np.ascontiguousarray(
            _q8(xs).reshape(KT, 128, BS).transpose(1, 0, 2)).reshape(128, -1)
        x16 = np.ascontiguousarray(
            xT.astype(np.float16).reshape(KT, 128, BS)
            .transpose(1, 0, 2)).reshape(128, -1)
        in_maps.append({
            "xhl": xhl, "x16": x16, "w16": w16, "m16": m16,
            "mu8": mu8, "mun": mun, "sig": sigma, "eye": eye,
        })
    return nc, in_maps


def kernel(x, weight, learnable_binary_mask, bias, mu, sigma,
           lit2conj, conj2form):
    inputs = {
        "x": x, "weight": weight,
        "learnable_binary_mask": learnable_binary_mask, "bias": bias,
        "mu": mu, "sigma": sigma, "lit2conj": lit2conj,
        "conj2form": conj2form,
    }
    nc, in_maps = prepare(inputs)
    res = bass_utils.run_bass_kernel_spmd(nc, in_maps,
                                          core_ids=list(range(NCORES)))
    out = np.concatenate(
        [res.results[i]["out"].reshape(128, NBT, F).transpose(1, 0, 2)
         .reshape(BS, F) for i in range(NCORES)], axis=0)
    return out.astype(np.float32)
